# revision 1
# baseline (speedup 1.0000x reference)
"""Baseline kernel (previous session) for A/B timing."""
import numpy as np

import concourse.bass as bass
import concourse.tile as tile
from concourse import bacc, mybir
from concourse.bass_utils import run_bass_kernel_spmd

F32 = mybir.dt.float32
F32R = mybir.dt.float32r
MULT = mybir.AluOpType.mult
ADD = mybir.AluOpType.add
EQ = mybir.AluOpType.is_equal

DEC_LO = np.array([-0.0105974018, 0.0328830117, 0.0308413818, -0.1870348117,
                   -0.0279837694, 0.6308807679, 0.7148465706, 0.2303778133], np.float32)
DEC_HI = np.array([-0.2303778133, 0.7148465706, -0.6308807679, -0.0279837694,
                   0.1870348117, 0.0308413818, -0.0328830117, -0.0105974018], np.float32)
REC_LO = DEC_LO[::-1].copy()
REC_HI = DEC_HI[::-1].copy()

L0, L1, L2, L3 = 8192, 4100, 2054, 1031
N_CORES = 8
ROWS_PER_CORE = 256
TILES_PER_CORE = 2

V, P, S, PE = "vector", "gpsimd", "scalar", "pe"

ASSIGN = {
    "d1": V, "a1": V, "d2": V, "a2": P, "d3": V, "a3": V,
    "b3e": V, "b3o": V,
    "b2s1e": PE, "b2s1o": PE, "b2s2e": PE, "b2s2o": PE,
    "b1s1e": PE, "b1s1o": PE, "b1s2e": PE, "b1s2o": PE, "b1s3e": PE, "b1s3o": PE,
    "b0s1e": PE, "b0s1o": PE, "b0s2e": PE, "b0s2o": PE, "b0s3e": PE, "b0s3o": PE,
}
PSUM_CHUNK = 512


class Ctx:
    def __init__(self, nc, pool, obpool, pspool, assign):
        self.nc = nc
        self.pool = pool
        self.obpool = obpool
        self.pspool = pspool
        self.assign = assign
        self.diag = {}

    def any_pe(self):
        return any(v == PE for v in self.assign.values())

    def build_consts(self):
        nc = self.nc
        ones = self.pool.tile([128, 128], F32, tag="ones")
        nc.vector.memset(ones[:], 1.0)
        ident = self.pool.tile([128, 128], F32, tag="ident")
        nc.gpsimd.affine_select(ident[:], ones[:], [[1, 128]], EQ, 0.0,
                                base=0, channel_multiplier=-1)
        vals = [float(np.float32(v)) for v in list(DEC_LO) + list(DEC_HI)]
        ll, hl = _get_u4_taps()
        cvals = set()
        for taps in (ll, hl):
            for r in range(4):
                cvals.update(float(v) for _, v in taps[r])
        vals = vals + sorted(cvals - set(vals))
        for i, w in enumerate(vals):
            d = self.pool.tile([128, 128], F32R, tag=f"diag{i}")
            nc.vector.tensor_scalar_mul(d[:], ident[:], float(w))
            self.diag[float(np.float32(w))] = d


def _interleave(*op_lists):
    n = max(len(l) for l in op_lists)
    for i in range(n):
        for l in op_lists:
            if i < len(l):
                l[i]()


def _ana_thunks(ctx, xp, out, w, L):
    nc = ctx.nc
    No = L // 2 + 4
    ops = []
    for k in range(8):
        src = xp[:, k:k + 2 * No - 1:2]
        if k == 0:
            ops.append(lambda o=out, s=src, v=float(w[0]): nc.scalar.mul(o, s, v))
        else:
            ops.append(lambda o=out, s=src, v=float(w[k]):
                       nc.vector.scalar_tensor_tensor(o, s, v, o, MULT, ADD))
    return ops


def _synth_phase_taps(w, phase):
    if phase == 0:
        return [(b, w[7 - 2 * b]) for b in range(4)]
    return [(c, w[8 - 2 * c]) for c in range(1, 5)]


def _emit_ana(ctx, xp, out, w, L, eng):
    nc = ctx.nc
    No = L // 2 + 4
    if eng == V:
        _interleave(_ana_thunks(ctx, xp, out, w, L))
    elif eng == P:
        tmp = ctx.pool.tile([128, No], F32, tag="ptmp")
        for k in range(8):
            src = xp[:, k:k + 2 * No - 1:2]
            if k == 0:
                nc.gpsimd.tensor_scalar_mul(out, src, float(w[0]))
            else:
                nc.gpsimd.tensor_scalar_mul(tmp[:, :No], src, float(w[k]))
                nc.gpsimd.tensor_tensor(out, out, tmp[:, :No], ADD)
    else:
        raise ValueError(eng)





def _compose_u4(w1, w2):
    """Taps of S_{w2}(S_{w1}(src)) as an up-4 map: out[4k+r] = sum w*src[k+off]."""
    out = {r: {} for r in range(4)}
    for r in range(4):
        p2 = r & 1
        c = (r - p2) // 2
        for off2, w2v in _synth_phase_taps(np.float64(w2), p2):
            t = c + off2
            p1 = t & 1
            q = (t - p1) // 2
            for off1, w1v in _synth_phase_taps(np.float64(w1), p1):
                out[r][q + off1] = out[r].get(q + off1, 0.0) + float(w2v) * float(w1v)
    return {r: sorted(out[r].items()) for r in out}


TAPS_LL = None
TAPS_HL = None


def _get_u4_taps():
    global TAPS_LL, TAPS_HL
    if TAPS_LL is None:
        TAPS_LL = _compose_u4(REC_LO, REC_LO)
        TAPS_HL = _compose_u4(REC_HI, REC_LO)
    return TAPS_LL, TAPS_HL


def _emit_synth_u4(ctx, x, dest, taps, H):
    """Fused two-stage synthesis on PE: dest[:, r::4] = sum w*x[:, k+off]."""
    nc = ctx.nc
    for r in range(4):
        for c0 in range(0, H, PSUM_CHUNK):
            n = min(PSUM_CHUNK, H - c0)
            ps = ctx.pspool.tile([128, PSUM_CHUNK], F32, tag="ps")
            tl = taps[r]
            for i, (off, wv) in enumerate(tl):
                rhs = x[:, c0 + off:c0 + off + n]
                nc.tensor.matmul(ps[:, :n], ctx.diag[float(np.float32(wv))][:], rhs,
                                 start=(i == 0), stop=(i == len(tl) - 1))
            s0 = r + 4 * c0
            nc.scalar.copy(dest[:, s0:s0 + 4 * (n - 1) + 1:4], ps[:, :n])


def _synth_v_thunks(ctx, x, dphase, taps, H):
    nc = ctx.nc
    ops = []
    for i, (off, wv) in enumerate(taps):
        src = x[:, off:off + H]
        if i == 0:
            ops.append(lambda o=dphase, s=src, v=float(wv): nc.scalar.mul(o, s, v))
        else:
            ops.append(lambda o=dphase, s=src, v=float(wv):
                       nc.vector.scalar_tensor_tensor(o, s, v, o, MULT, ADD))
    return ops


def _emit_synth_phase(ctx, x, dest, w, T, phase, eng):
    nc = ctx.nc
    H = T // 2
    taps = _synth_phase_taps(w, phase)
    dphase = dest[:, phase:T:2]
    if eng == V:
        _interleave(_synth_v_thunks(ctx, x, dphase, taps, H))
    elif eng == PE:
        He = H - (H % 2)
        for c0 in range(0, He, PSUM_CHUNK):
            n = min(PSUM_CHUNK, He - c0)
            ps = ctx.pspool.tile([128, PSUM_CHUNK], F32, tag="ps")
            for i, (off, wv) in enumerate(taps):
                rhs = x[:, c0 + off:c0 + off + n]
                nc.tensor.matmul(ps[:, :n], ctx.diag[float(np.float32(wv))][:],
                                 rhs, start=(i == 0), stop=(i == 3))
            s0 = phase + 2 * c0
            nc.scalar.copy(dest[:, s0:s0 + 2 * n - 1:2], ps[:, :n])
        if He < H:
            c0 = He
            dcol = dest[:, phase + 2 * c0:phase + 2 * c0 + 1]
            for i, (off, wv) in enumerate(taps):
                src = x[:, c0 + off:c0 + off + 1]
                if i == 0:
                    nc.scalar.mul(dcol, src, float(wv))
                else:
                    nc.vector.scalar_tensor_tensor(dcol, src, float(wv), dcol,
                                                   MULT, ADD)
    else:
        raise ValueError(eng)


def _emit_synth(ctx, x, dest, w, T, key):
    _emit_synth_phase(ctx, x, dest, w, T, 0, ctx.assign[key + "e"])
    _emit_synth_phase(ctx, x, dest, w, T, 1, ctx.assign[key + "o"])


def _emit_reflect(ctx, xp, L):
    nc = ctx.nc
    nc.vector.tensor_copy(xp[:, 0:7], xp[:, 14:7:-1])
    nc.vector.tensor_copy(xp[:, 7 + L:14 + L], xp[:, L + 5:L - 2:-1])


def build_nc(assign=None):
    a = dict(ASSIGN)
    if assign:
        a.update(assign)
    nc = bacc.Bacc("TRN2", target_bir_lowering=False, debug=False,
                   num_devices=N_CORES)
    x_ap = nc.dram_tensor("x", [ROWS_PER_CORE, L0], F32, kind="ExternalInput").ap()
    y_ap = nc.dram_tensor("y", [4, ROWS_PER_CORE, L0], F32, kind="ExternalOutput").ap()

    with tile.TileContext(nc) as tc:
        with tc.tile_pool(name="bufs", bufs=1) as pool, \
             tc.tile_pool(name="ob", bufs=2) as obpool, \
             tc.tile_pool(name="ps", bufs=8, space="PSUM") as pspool:
            ctx = Ctx(nc, pool, obpool, pspool, a)
            ctx.build_consts()
            tile_over = [
                {"b3e": PE, "b3o": PE},
                {"b0s3e": V, "b0s3o": V},
            ]
            base_assign = dict(a)

            for t in range(TILES_PER_CORE):
                a = dict(base_assign)
                a.update(tile_over[t % len(tile_over)])
                ctx.assign = a

                def syn_dt(key):
                    return F32R if (a[key + "e"] == PE or a[key + "o"] == PE) else F32

                rows = slice(t * 128, (t + 1) * 128)

                xp0 = pool.tile([128, L0 + 14], F32, tag="xp0")
                if a["b3e"] == PE:
                    nc.sync.dma_start(xp0[:, 7:7 + 4800], x_ap[rows, 0:4800])
                    nc.sync.dma_start(xp0[:, 7 + 4800:7 + L0], x_ap[rows, 4800:L0])
                else:
                    nc.sync.dma_start(xp0[:, 7:7 + L0], x_ap[rows, :])
                _emit_reflect(ctx, xp0, L0)

                d1 = pool.tile([128, L1], syn_dt("b3"), tag="d1")
                a1p = pool.tile([128, L1 + 14], F32, tag="a1p")
                if a["b3e"] == PE:
                    Hh = L1 // 2
                    for k in range(8):
                        src = xp0[:, k:k + 2 * Hh - 1:2]
                        dst = d1[:, 0:Hh]
                        if k == 0:
                            nc.scalar.mul(dst, src, float(DEC_HI[0]))
                        else:
                            nc.vector.scalar_tensor_tensor(
                                dst, src, float(DEC_HI[k]), dst, MULT, ADD)
                    n2 = L1 - Hh
                    for k in range(8):
                        src = xp0[:, 2 * Hh + k:2 * Hh + k + 2 * n2 - 1:2]
                        dst = d1[:, Hh:L1]
                        if k == 0:
                            nc.scalar.mul(dst, src, float(DEC_HI[0]))
                        else:
                            nc.vector.scalar_tensor_tensor(
                                dst, src, float(DEC_HI[k]), dst, MULT, ADD)
                    _emit_ana(ctx, xp0, a1p[:, 7:7 + L1], DEC_LO, L0, a["a1"])
                elif a["d1"] == V and a["a1"] == V:
                    _interleave(_ana_thunks(ctx, xp0, d1[:], DEC_HI, L0),
                                _ana_thunks(ctx, xp0, a1p[:, 7:7 + L1], DEC_LO, L0))
                else:
                    _emit_ana(ctx, xp0, d1[:], DEC_HI, L0, a["d1"])
                    _emit_ana(ctx, xp0, a1p[:, 7:7 + L1], DEC_LO, L0, a["a1"])
                _emit_reflect(ctx, a1p, L1)

                def emit_b3(d1=d1, rows=rows):
                    ob3 = obpool.tile([128, L0], F32, tag="ob")
                    if a["b3e"] == V and a["b3o"] == V:
                        _interleave(
                            _synth_v_thunks(ctx, d1, ob3[:, 0:L0:2],
                                            _synth_phase_taps(REC_HI, 0), L0 // 2),
                            _synth_v_thunks(ctx, d1, ob3[:, 1:L0:2],
                                            _synth_phase_taps(REC_HI, 1), L0 // 2))
                    else:
                        _emit_synth(ctx, d1, ob3, REC_HI, L0, "b3")
                    nc.sync.dma_start(y_ap[3, rows, :], ob3[:])

                if a["b3e"] == PE:
                    emit_b3()

                d2 = pool.tile([128, L2], syn_dt("b2s1"), tag="d2")
                _emit_ana(ctx, a1p, d2[:], DEC_HI, L1, a["d2"])
                a2p = pool.tile([128, L2 + 14], F32, tag="a2p")
                _emit_ana(ctx, a1p, a2p[:, 7:7 + L2], DEC_LO, L1, a["a2"])
                _emit_reflect(ctx, a2p, L2)

                ob2 = obpool.tile([128, L0], F32, tag="ob")
                _emit_synth_u4(ctx, d2, ob2, _get_u4_taps()[1], L0 // 4)
                nc.sync.dma_start(y_ap[2, rows, :], ob2[:])

                d3 = pool.tile([128, L3], syn_dt("b1s1"), tag="d3")
                a3 = pool.tile([128, L3], syn_dt("b0s1"), tag="a3")
                if a["d3"] == V and a["a3"] == V:
                    _interleave(_ana_thunks(ctx, a2p, d3[:], DEC_HI, L2),
                                _ana_thunks(ctx, a2p, a3[:], DEC_LO, L2))
                else:
                    _emit_ana(ctx, a2p, d3[:], DEC_HI, L2, a["d3"])
                    _emit_ana(ctx, a2p, a3[:], DEC_LO, L2, a["a3"])

                if a["b3e"] != PE:
                    emit_b3()

                u_ = pool.tile([128, L2], syn_dt("b1s2"), tag="u")
                _emit_synth(ctx, d3, u_, REC_HI, L2, "b1s1")
                ob1 = obpool.tile([128, L0], F32, tag="ob")
                _emit_synth_u4(ctx, u_, ob1, _get_u4_taps()[0], L0 // 4)
                nc.sync.dma_start(y_ap[1, rows, :], ob1[:])

                u_ = pool.tile([128, L2], syn_dt("b0s2"), tag="u")
                _emit_synth(ctx, a3, u_, REC_LO, L2, "b0s1")
                ob0 = obpool.tile([128, L0], F32, tag="ob")
                _emit_synth_u4(ctx, u_, ob0, _get_u4_taps()[0], L0 // 4)
                nc.sync.dma_start(y_ap[0, rows, :], ob0[:])

    nc.compile()
    return nc


_NC = None


def _get_nc():
    global _NC
    if _NC is None:
        _NC = build_nc()
    return _NC


def shard_inputs(x):
    rows = np.ascontiguousarray(x.reshape(-1, L0))
    return [{"x": rows[c * ROWS_PER_CORE:(c + 1) * ROWS_PER_CORE]}
            for c in range(N_CORES)]


def unshard_outputs(results):
    out = np.empty((4, N_CORES * ROWS_PER_CORE, L0), np.float32)
    for c, r in enumerate(results):
        out[:, c * ROWS_PER_CORE:(c + 1) * ROWS_PER_CORE, :] = r["y"]
    return out.reshape(4, 16, 128, L0)


def kernel(x):
    x = np.asarray(x, np.float32)
    assert x.shape == (16, 128, L0), x.shape
    nc = _get_nc()
    res = run_bass_kernel_spmd(nc, shard_inputs(x), core_ids=list(range(N_CORES)))
    return unshard_outputs(res.results)



# revision 12
# speedup vs baseline: 2.1968x; 2.1968x over previous
"""DWT front-end as fused banded matmuls on the PE array.

Each output band is a linear map of x (reflect-pad + conv cascades + crops all
compose into one banded matrix per band). Per 128-sample position tile the map
is block-tridiagonal; interior blocks repeat every 128 positions, so only ~20
unique 128x128 fp16 blocks exist across all 4 bands. The kernel streams
x^T tiles (built on host) through the PE as lhsT and the weight blocks as rhs,
accumulating row-major [128 rows, 512 outs] chunks in PSUM, then evacuates to
fp16 SBUF and DMAs out.
"""
import numpy as np
from numpy.lib.stride_tricks import sliding_window_view

import concourse.bass as bass
import concourse.tile as tile
from concourse import bacc, mybir
from concourse.bass_utils import run_bass_kernel_spmd

F32 = mybir.dt.float32
F16 = mybir.dt.float16

LEVELS = 3
K = 8
DEC_LO = np.array([-0.0105974018, 0.0328830117, 0.0308413818, -0.1870348117,
                   -0.0279837694, 0.6308807679, 0.7148465706, 0.2303778133], np.float64)
DEC_HI = np.array([-0.2303778133, 0.7148465706, -0.6308807679, -0.0279837694,
                   0.1870348117, 0.0308413818, -0.0328830117, -0.0105974018], np.float64)
REC_LO = np.array([0.2303778133, 0.7148465706, 0.6308807679, -0.0279837694,
                   -0.1870348117, 0.0308413818, 0.0328830117, -0.0105974018], np.float64)
REC_HI = np.array([-0.0105974018, -0.0328830117, 0.0308413818, 0.1870348117,
                   -0.0279837694, -0.6308807679, 0.7148465706, -0.2303778133], np.float64)

L0 = 8192
N_CORES = 8
ROWS_PER_CORE = 256
T = 128            # position tile (matmul contraction)
NT = L0 // T       # 64
CW = 512           # psum chunk width (one bank)
NG = L0 // CW      # 16 chunks per band


# ---------------------------------------------------------------- host math
def _dwconv_s2(x, w):
    xp = np.pad(x, ((0, 0), (K - 1, K - 1)), mode="reflect")
    win = sliding_window_view(xp, K, axis=1)[:, ::2]
    return win @ w


def _dwconvT_s2(x, w):
    n, L = x.shape
    xd = np.zeros((n, 2 * L - 1 + 2 * (K - 1)), x.dtype)
    xd[:, K - 1:K - 1 + 2 * L - 1:2] = x
    win = sliding_window_view(xd, K, axis=1)
    return win @ w[::-1]


def _fit(out, target_len):
    L = out.shape[-1]
    if L > target_len:
        s = (L - target_len) // 2
        return out[:, s:s + target_len]
    if L < target_len:
        return np.pad(out, ((0, 0), (0, target_len - L)))
    return out


def _band_pipeline(x):
    approx = x
    details = []
    target_lens = []
    for _ in range(LEVELS):
        target_lens.append(approx.shape[-1])
        details.append(_dwconv_s2(approx, DEC_HI))
        approx = _dwconv_s2(approx, DEC_LO)

    def recon(band_idx):
        if band_idx == 0:
            rec = approx
            for lvl in reversed(range(LEVELS)):
                rec = _fit(_dwconvT_s2(rec, REC_LO), target_lens[lvl])
            return rec
        pick = LEVELS - band_idx
        rec = _fit(_dwconvT_s2(details[pick], REC_HI), target_lens[pick])
        for lvl in reversed(range(pick)):
            rec = _fit(_dwconvT_s2(rec, REC_LO), target_lens[lvl])
        return rec

    return np.stack([_fit(recon(i), L0) for i in range(LEVELS + 1)], axis=0)


def _build_R():
    Rs = [np.zeros((L0, L0), np.float32) for _ in range(4)]
    chunk = 2048
    for s in range(0, L0, chunk):
        I = np.zeros((chunk, L0), np.float32)
        I[np.arange(chunk), s + np.arange(chunk)] = 1.0
        out = _band_pipeline(I)
        for b in range(4):
            Rs[b][s:s + chunk] = out[b]
    return Rs


def _build_schedule():
    """Derive weight blocks + per-(t) matmul segments.

    Returns (wmat, segs_by_t, closes, first_touch) where
      wmat: [128, 128*nblk] fp16 weight matrix (canonical blocks)
      segs_by_t[t]: list of (b, g, pcol_lo, pcol_hi, wcol_lo, wcol_hi,
                             start, stop) matmul segments for position tile t
      closes[t]: list of (b, g) psum groups whose last matmul is at tile t
      first_touch[(b, g)]: first t touching the group
    """
    Rs = _build_R()
    blocks = []
    bmap = {}
    # per band, per 128-col chunk c: instances {t: (blk, span_lo, span_hi)}
    inst = [[dict() for _ in range(NT)] for _ in range(4)]
    for b in range(4):
        R = Rs[b]
        for c in range(NT):
            cols = R[:, c * T:(c + 1) * T]
            for t in range(NT):
                blk = cols[t * T:(t + 1) * T]
                if not np.any(blk):
                    continue
                h = np.float16(blk).tobytes()
                idx = bmap.get(h)
                if idx is None:
                    idx = len(blocks)
                    blocks.append(np.float16(blk))
                    bmap[h] = idx
                inst[b][c][t] = idx
    del Rs

    segs_by_t = [[] for _ in range(NT)]
    closes = [[] for _ in range(NT)]
    first_touch = {}
    for b in range(4):
        for c in range(NT):
            ts = sorted(inst[b][c])
            blk_of = inst[b][c]
            # per-column tmin/tmax within this 128-col chunk
            tmin = np.full(T, 10 ** 9, np.int64)
            tmax = np.full(T, -1, np.int64)
            for t in ts:
                blk = blocks[blk_of[t]]
                nz = np.any(blk != 0, axis=0)
                idxs = np.flatnonzero(nz)
                tmin[idxs] = np.minimum(tmin[idxs], t)
                tmax[idxs] = np.maximum(tmax[idxs], t)
            assert (tmax >= 0).all(), f"uncovered column in band {b} chunk {c}"
            g = c // (CW // T)
            key = (b, g)
            for t in ts:
                blk = blocks[blk_of[t]]
                nz = np.any(blk != 0, axis=0)
                # maximal runs of nonzero columns, split where first-touch
                # status changes (psum pending-zero is all-or-none per inst)
                j = 0
                while j < T:
                    if not nz[j]:
                        j += 1
                        continue
                    st = bool(tmin[j] == t)
                    j2 = j
                    while (j2 < T and nz[j2] and (tmin[j2] == t) == st):
                        j2 += 1
                    pcol = (c * T - g * CW) + j
                    wcol = blk_of[t] * T + j
                    segs_by_t[t].append((b, g, pcol, pcol + (j2 - j),
                                         wcol, wcol + (j2 - j), st, False))
                    j = j2
                if key not in first_touch:
                    first_touch[key] = t
            # group close: last t over the group's chunks
    last_t = {}
    for t in range(NT):
        for (b, g, *_rest) in segs_by_t[t]:
            last_t[(b, g)] = t
    for (b, g), t in last_t.items():
        closes[t].append((b, g))
    wmat = np.concatenate(blocks, axis=1)  # [128, nblk*128]
    return wmat, segs_by_t, closes, first_touch


_SCHED = None


def _get_sched():
    global _SCHED
    if _SCHED is None:
        _SCHED = _build_schedule()
    return _SCHED


# ---------------------------------------------------------------- bass build
def build_nc():
    wmat, segs_by_t, closes, first_touch = _get_sched()
    nblk_cols = wmat.shape[1]

    nc = bacc.Bacc("TRN2", target_bir_lowering=False, debug=False,
                   num_devices=N_CORES)
    xt_ap = nc.dram_tensor("xt", [L0, ROWS_PER_CORE], F16,
                           kind="ExternalInput").ap()
    w_ap = nc.dram_tensor("w", [T, nblk_cols], F16, kind="ExternalInput").ap()
    y_ap = nc.dram_tensor("y", [4, ROWS_PER_CORE, L0], F16,
                          kind="ExternalOutput").ap()

    with tile.TileContext(nc) as tc:
        with tc.tile_pool(name="bufs", bufs=1) as pool, \
             tc.tile_pool(name="ob", bufs=2) as obpool, \
             tc.tile_pool(name="ps", bufs=1, space="PSUM") as pspool:
            w_sb = pool.tile([T, nblk_cols], F16, tag="w")
            nc.scalar.dma_start(w_sb[:], w_ap[:, :])
            xt_sb = pool.tile([128, NT * ROWS_PER_CORE], F16, tag="xt")
            xt_src = xt_ap.rearrange("(t p) r -> p t r", p=128)
            HT = NT // 4
            for q in range(4):
                nc.sync.dma_start(
                    xt_sb[:, q * HT * ROWS_PER_CORE:(q + 1) * HT * ROWS_PER_CORE],
                    xt_src[:, q * HT:(q + 1) * HT, :])

            evac_engines = [nc.vector, nc.scalar]
            ev_i = 0
            dma_q = [nc.sync, nc.gpsimd]

            # psum start/stop are per 2KB zero region (the whole bank):
            # exactly one start (first matmul into the bank) and one stop
            # (last matmul) per (b, g) group.
            last_seg = {}
            for t in range(NT):
                for i, (b, g, *_r) in enumerate(segs_by_t[t]):
                    last_seg[(b, g)] = (t, i)

            for rt in range(2):
                y_sb = []
                for b in range(4):
                    yb = obpool.tile([128, L0], F16, tag=f"y{b}", name=f"y{b}")
                    y_sb.append(yb)
                ps_tiles = {}
                done_groups = [0] * 4
                for t in range(NT):
                    lhsT = xt_sb[:, t * ROWS_PER_CORE + rt * 128:
                                 t * ROWS_PER_CORE + rt * 128 + 128]
                    for i, (b, g, plo, phi, wlo, whi, _st, _sp) in enumerate(
                            segs_by_t[t]):
                        key = (b, g)
                        first = key not in ps_tiles
                        if first:
                            ps_tiles[key] = pspool.tile(
                                [128, CW], F32, tag=f"ps{b}_{g % 2}",
                                name=f"ps{b}_{g % 2}")
                        nc.tensor.matmul(ps_tiles[key][:, plo:phi], lhsT,
                                         w_sb[:, wlo:whi], start=first,
                                         stop=last_seg[key] == (t, i))
                    for (b, g) in closes[t]:
                        eng = evac_engines[ev_i % 2]
                        ev_i += 1
                        dst = y_sb[b][:, g * CW:(g + 1) * CW]
                        src = ps_tiles.pop((b, g))[:]
                        if eng is nc.scalar:
                            eng.copy(dst, src)
                        else:
                            eng.tensor_copy(dst, src)
                        done_groups[b] += 1
                        if done_groups[b] == NG:
                            rows = slice(rt * 128, (rt + 1) * 128)
                            dma_q[(b + rt) % 2].dma_start(
                                y_ap[b, rows, :], y_sb[b][:])
    nc.compile()
    return nc


_NC = None


def _get_nc():
    global _NC
    if _NC is None:
        _NC = build_nc()
    return _NC


def shard_inputs(x):
    wmat = _get_sched()[0]
    rows = np.ascontiguousarray(x.reshape(-1, L0))
    out = []
    for c in range(N_CORES):
        shard = rows[c * ROWS_PER_CORE:(c + 1) * ROWS_PER_CORE]
        xt = np.ascontiguousarray(shard.astype(np.float16).T)
        out.append({"xt": xt, "w": wmat})
    return out


def unshard_outputs(results):
    out = np.empty((4, N_CORES * ROWS_PER_CORE, L0), np.float32)
    for c, r in enumerate(results):
        out[:, c * ROWS_PER_CORE:(c + 1) * ROWS_PER_CORE, :] = r["y"]
    return out.reshape(4, 16, 128, L0)


def kernel(x):
    x = np.asarray(x, np.float32)
    assert x.shape == (16, 128, L0), x.shape
    nc = _get_nc()
    res = run_bass_kernel_spmd(nc, shard_inputs(x), core_ids=list(range(N_CORES)))
    return unshard_outputs(res.results)
